# revision 1
# baseline (speedup 1.0000x reference)
"""Trainium2 Bass kernel for nn_LogicGatedSpikingSelfAttention.

Sharding: channel/head-parallel over 8 cores. Each core owns 128 output
channels = 2 heads for the q/k/v branches (BN stats fully local, since
stats are per-channel over all tokens), runs attention for its 2 heads
over all 4 batches locally, and computes a 128-output-channel slice of
the projection. One AllGather moves the binary attention spikes (+ per-
head energies for the logic gate) between the attention and projection
stages; the gate is folded into the projection weights after the gather
(exact: gate is {0,1}).

Numerics: all big matmuls in bf16. The attention is exact in integers
(spikes are {0,1}: counts accumulate exactly in fp32 PSUM, and the
attn-LIF threshold reduces to an integer compare S >= 2^0.75). The LIF
forward pass is a pure Heaviside, so each branch reduces to
Y >= m + (2-beta)/gamma * sqrt(var+eps) with per-channel scalars.
"""
import numpy as np
import ml_dtypes

import concourse.bass as bass
import concourse.bacc as bacc
import concourse.tile as tile
from concourse import mybir
from concourse.bass_utils import run_bass_kernel_spmd

NCORES = 8
B, NSEQ, D, H = 4, 1024, 1024, 16
HD = D // H            # 64 head dim
CH = D // NCORES       # 128 channels per core
TOK = B * NSEQ         # 4096 tokens
KT = D // 128          # 8 contraction tiles
EPS = 1e-5
S_TH = float(2.0 ** 0.75)   # x_attn >= 1  <=>  S >= hd**0.125 = 2^0.75
SPIKE_N = 128 * TOK         # flat payload: spikes then 8 energy slots
PAYLEN = SPIKE_N + 8
F32 = mybir.dt.float32
BF16 = mybir.dt.bfloat16
BF = ml_dtypes.bfloat16

_CACHE = {}


def _build():
    nc = bacc.Bacc("TRN2", target_bir_lowering=False, debug=False,
                   num_devices=NCORES)
    inp = {}
    def din(name, shape, dt=BF16):
        inp[name] = nc.dram_tensor(name, shape, dt, kind="ExternalInput")
        return inp[name]

    din("xT",  [128, KT * TOK])          # host pre-tiled: [p, (t n)]
    din("wq",  [128, KT * CH]); din("wk", [128, KT * CH])
    din("wv",  [128, KT * CH]); din("wp", [128, KT * CH])
    for nm in ("tq", "tk", "tv", "tp", "bq", "bk", "bv", "bp"):
        din(nm, [CH, 1], F32)
    din("wgr", [H, H], F32)              # lhsT: [h, h'] = sum_r Wg[h', h+16r]/1024
    din("bgr", [H, 1], F32)
    din("i2e", [CH, 2], F32)             # [p, j] = (p//64==j)
    din("i16", [H, KT * 128], F32)       # [h, (t m)] = (t*128+m)//64 == h
    din("idn", [128, 128])               # identity for PE transpose
    outT = nc.dram_tensor("outT", [CH, TOK], BF16, kind="ExternalOutput")

    with tile.TileContext(nc) as tc:
        with tc.tile_pool(name="consts", bufs=1) as consts, \
             tc.tile_pool(name="spikes", bufs=1) as spk_pool, \
             tc.tile_pool(name="dram", bufs=1, space="DRAM") as dram:
            _body(tc, inp, outT, consts, spk_pool, dram)
    nc.compile()
    return nc


def _body(tc, inp, outT, consts, spk_pool, dram):
    nc = tc.nc
    V, SC, GP, TE = nc.vector, nc.scalar, nc.gpsimd, nc.tensor
    AF = mybir.ActivationFunctionType
    OP = mybir.AluOpType
    DENG = [nc.sync, nc.scalar, nc.gpsimd]

    # ---- constants / weights to SBUF (all host-contiguous) ----
    w_sb = {}
    for i, nm in enumerate(("wq", "wk", "wv", "wp")):
        t = consts.tile([128, KT, CH], BF16, name=f"{nm}_sb")
        DENG[i % 3].dma_start(
            t[:], inp[nm].ap().rearrange("p (t m) -> p t m", t=KT))
        w_sb[nm] = t
    small = {}
    for nm in ("tq", "tk", "tv", "tp", "bq", "bk", "bv", "bp", "bgr"):
        t = consts.tile([inp[nm].shape[0], 1], F32, name=f"{nm}_sb")
        nc.sync.dma_start(t[:], inp[nm].ap())
        small[nm] = t
    wgr_sb = consts.tile([H, H], F32)
    nc.sync.dma_start(wgr_sb[:], inp["wgr"].ap())
    i2e_sb = consts.tile([CH, 2], F32)
    nc.sync.dma_start(i2e_sb[:], inp["i2e"].ap())
    i16_sb = consts.tile([H, KT, 128], F32)
    nc.sync.dma_start(i16_sb[:],
                      inp["i16"].ap().rearrange("h (t m) -> h t m", t=KT))
    idn_sb = consts.tile([128, 128], BF16)
    nc.scalar.dma_start(idn_sb[:], inp["idn"].ap())
    eps_sb = consts.tile([128, 1], F32)
    V.memset(eps_sb[:], EPS)

    # ---- persistent spike tensors ----
    spA = {nm: spk_pool.tile([128, TOK], BF16, name=f"sp{nm}A")
           for nm in ("q", "k", "v")}
    sp2 = {nm: spk_pool.tile([HD, 2, TOK], BF16, name=f"sp2{nm}")
           for nm in ("q", "k")}
    vnat = spk_pool.tile([128, 32, 128], BF16)          # [tok, b*8+mt, ch]
    payload = spk_pool.tile([HD, 2, TOK], BF16)         # [d, h, tok] spikes

    # ================= branches (q, k, v) =================
    with tc.tile_pool(name="xts_p", bufs=1) as xts_p, \
         tc.tile_pool(name="ybig", bufs=2) as ybig, \
         tc.tile_pool(name="stps", bufs=2) as stp:
        xts = xts_p.tile([128, KT, TOK], BF16)
        nc.gpsimd.dma_start(
            xts[:], inp["xT"].ap().rearrange("p (t n) -> p t n", t=KT))

        for nm in ("q", "k", "v"):
            Y = ybig.tile([128, TOK], F32, tag="Y")
            # weight-stationary: kt outer, 8 PSUM banks accumulate
            with tc.tile_pool(name=f"brps_{nm}", bufs=1, space="PSUM") as brps:
                ps = [brps.tile([128, 512], F32, name=f"ps{nm}{i}")
                      for i in range(8)]
                for kt in range(KT):
                    for nck in range(8):
                        TE.matmul(ps[nck][:], w_sb["w" + nm][:, kt, :],
                                  xts[:, kt, nck * 512:(nck + 1) * 512],
                                  start=(kt == 0), stop=(kt == KT - 1))
                for nck in range(8):
                    if nck % 2:
                        V.tensor_scalar(Y[:, nck * 512:(nck + 1) * 512],
                                        ps[nck][:], small["b" + nm][:],
                                        None, OP.add)
                    else:
                        SC.activation(Y[:, nck * 512:(nck + 1) * 512],
                                      ps[nck][:], AF.Identity,
                                      bias=small["b" + nm][:])
            stats = stp.tile([128, 8, 6], F32, tag="stats")
            for i in range(8):
                V.bn_stats(stats[:, i, :], Y[:, i * 512:(i + 1) * 512])
            mv = stp.tile([128, 2], F32, tag="mv")
            V.bn_aggr(mv[:], stats[:])
            std = stp.tile([128, 1], F32, tag="std")
            SC.activation(std[:], mv[:, 1:2], AF.Sqrt, bias=eps_sb[:])
            thr = stp.tile([128, 1], F32, tag="thr")
            V.tensor_tensor(thr[:], std[:], small["t" + nm][:], OP.mult)
            V.tensor_tensor(thr[:], thr[:], mv[:, 0:1], OP.add)
            V.tensor_scalar(spA[nm][:], Y[:], thr[:], None, OP.is_ge)

        # head-split q, k for attention operand layout (base partition 0)
        for i, nm in enumerate(("q", "k")):
            for h in range(2):
                DENG[(i * 2 + h) % 3].dma_start(
                    sp2[nm][:, h, :], spA[nm][h * HD:(h + 1) * HD, :])

        # v -> natural (token, channel) layout via PE transpose
        with tc.tile_pool(name="tps", bufs=4, space="PSUM") as tps:
            for i in range(32):
                vt = tps.tile([128, 128], BF16, tag="vt")
                TE.transpose(vt[:], spA["v"][:, i * 128:(i + 1) * 128], idn_sb[:])
                if i % 2 == 0:
                    V.tensor_copy(vnat[:, i, :], vt[:])
                else:
                    SC.activation(vnat[:, i, :], vt[:], AF.Copy)

    # ================= energy =================
    e_sb = spk_pool.tile([2, B], BF16)
    with tc.tile_pool(name="enps", bufs=1, space="PSUM") as enps, \
         tc.tile_pool(name="entmp", bufs=1) as entmp:
        prod = entmp.tile([128, TOK], BF16)
        V.tensor_tensor(prod[:], spA["q"][:], spA["k"][:], OP.mult)
        ech = entmp.tile([128, B], F32)
        V.reduce_sum(ech[:], prod[:].rearrange("p (b n) -> p b n", b=B),
                     axis=mybir.AxisListType.X)
        e_ps = enps.tile([2, B], F32)
        TE.matmul(e_ps[:], i2e_sb[:], ech[:], start=True, stop=True)
        V.tensor_copy(e_sb[:], e_ps[:])

    # ================= attention =================
    with tc.tile_pool(name="cps", bufs=3, space="PSUM") as cps, \
         tc.tile_pool(name="sps", bufs=4, space="PSUM") as sps, \
         tc.tile_pool(name="csb", bufs=4) as csb:
        for b in range(B):
            for h in range(2):
                s_ps = [sps.tile([HD, 512], F32, tag="sps", name=f"s_ps{b}{h}{i}")
                        for i in range(2)]
                for mt in range(8):
                    m0 = b * NSEQ + mt * 128
                    for ncn in range(2):
                        n0 = b * NSEQ + ncn * 512
                        c_ps = cps.tile([128, 512], F32, tag="cps")
                        TE.matmul(c_ps[:], sp2["k"][:, h, m0:m0 + 128],
                                  sp2["q"][:, h, n0:n0 + 512],
                                  start=True, stop=True)
                        c_sb = csb.tile([128, 512], BF16, tag="csb")
                        if (mt * 2 + ncn) % 4 == 3:
                            SC.activation(c_sb[:], c_ps[:], AF.Copy)
                        else:
                            V.tensor_copy(c_sb[:], c_ps[:])
                        TE.matmul(s_ps[ncn][:],
                                  vnat[:, b * 8 + mt, h * HD:(h + 1) * HD],
                                  c_sb[:], start=(mt == 0), stop=(mt == 7))
                for ncn in range(2):
                    n0 = b * NSEQ + ncn * 512
                    V.tensor_scalar(payload[:, h, n0:n0 + 512], s_ps[ncn][:],
                                    S_TH, None, OP.is_ge)

    # ================= AllGather (flat, contiguous) =================
    pay_d = dram.tile([PAYLEN], BF16)
    gath_d = dram.tile([NCORES, PAYLEN], BF16, addr_space="Shared")
    for h in range(2):
        DENG[h].dma_start(
            pay_d[h * HD * TOK:(h + 1) * HD * TOK].rearrange(
                "(p n) -> p n", p=HD),
            payload[:, h, :])
    nc.sync.dma_start(
        pay_d[SPIKE_N:SPIKE_N + 8].rearrange("(p w) -> p w", p=2), e_sb[:])
    GP.collective_compute("AllGather", OP.bypass,
                          ins=[pay_d.opt()], outs=[gath_d.opt()],
                          replica_groups=[list(range(NCORES))])

    # ================= gate -> gated proj weights =================
    with tc.tile_pool(name="gtmp", bufs=1) as gtmp, \
         tc.tile_pool(name="post", bufs=1) as post, \
         tc.tile_pool(name="pstat", bufs=1) as pstat:
        with tc.tile_pool(name="gtps", bufs=2, space="PSUM") as gtps:
            eg_bf = gtmp.tile([H, B], BF16)
            nc.sync.dma_start(
                eg_bf[:],
                gath_d[:, SPIKE_N:SPIKE_N + 8].rearrange(
                    "c (p w) -> c p w", p=2))
            eg = gtmp.tile([H, B], F32)
            V.tensor_copy(eg[:], eg_bf[:])
            g_ps = gtps.tile([H, B], F32, tag="gps")
            TE.matmul(g_ps[:], wgr_sb[:], eg[:], start=True, stop=True)
            gate = gtmp.tile([H, B], F32)
            V.tensor_scalar(gate[:], g_ps[:], small["bgr"][:], 0.5,
                            OP.add, OP.is_ge)
            gv = gtmp.tile([128, KT, B], F32)
            for t in range(KT):
                gv_ps = gtps.tile([128, B], F32, tag="gvps")
                TE.matmul(gv_ps[:], i16_sb[:, t, :], gate[:],
                          start=True, stop=True)
                V.tensor_copy(gv[:, t, :], gv_ps[:])
            wpg = post.tile([128, KT, B, 128], BF16)
            for t in range(KT):
                for b in range(B):
                    if (t * B + b) % 2:
                        V.tensor_scalar(wpg[:, t, b, :], w_sb["wp"][:, t, :],
                                        gv[:, t, b:b + 1], None, OP.mult)
                    else:
                        SC.activation(wpg[:, t, b, :], w_sb["wp"][:, t, :],
                                      AF.Identity, scale=gv[:, t, b:b + 1])

        # ================= projection =================
        rhs = [post.tile([128, TOK], BF16, name=f"rhs{t}") for t in range(KT)]
        for t in range(KT):
            DENG[t % 3].dma_start(
                rhs[t][:],
                gath_d[t, 0:SPIKE_N].rearrange("(p n) -> p n", p=128))
        Yp = post.tile([128, TOK], F32)
        with tc.tile_pool(name="ppps", bufs=1, space="PSUM") as ppps:
            pp = [ppps.tile([128, 512], F32, name=f"pp{i}") for i in range(8)]
            for t in range(KT):
                for b in range(B):
                    for ncn in range(2):
                        n0 = b * NSEQ + ncn * 512
                        TE.matmul(pp[b * 2 + ncn][:], wpg[:, t, b, :],
                                  rhs[t][:, n0:n0 + 512],
                                  start=(t == 0), stop=(t == KT - 1))
            for i in range(8):
                if i % 2:
                    V.tensor_scalar(Yp[:, i * 512:(i + 1) * 512], pp[i][:],
                                    small["bp"][:], None, OP.add)
                else:
                    SC.activation(Yp[:, i * 512:(i + 1) * 512], pp[i][:],
                                  AF.Identity, bias=small["bp"][:])
        stats = pstat.tile([128, 8, 6], F32)
        for i in range(8):
            V.bn_stats(stats[:, i, :], Yp[:, i * 512:(i + 1) * 512])
        mv = pstat.tile([128, 2], F32)
        V.bn_aggr(mv[:], stats[:])
        std = pstat.tile([128, 1], F32)
        SC.activation(std[:], mv[:, 1:2], AF.Sqrt, bias=eps_sb[:])
        thr = pstat.tile([128, 1], F32)
        V.tensor_tensor(thr[:], std[:], small["tp"][:], OP.mult)
        V.tensor_tensor(thr[:], thr[:], mv[:, 0:1], OP.add)
        osb = pstat.tile([128, TOK], BF16)
        V.tensor_scalar(osb[:], Yp[:], thr[:], None, OP.is_ge)
        nc.sync.dma_start(outT.ap(), osb[:])


def _tile_rows(a):
    # (8*128, N) -> (128, 8*N) so the SBUF [p, (t n)] load is contiguous
    n = a.shape[1]
    return np.ascontiguousarray(
        a.reshape(KT, 128, n).transpose(1, 0, 2).reshape(128, KT * n))


def _prep_inputs(inputs):
    x = np.asarray(inputs["x"], np.float32)
    xT = _tile_rows(x.reshape(TOK, D).T.astype(BF))
    Wg = np.asarray(inputs["Wg"], np.float64)
    wgr = (Wg.reshape(H, HD, H).sum(axis=1).T / 1024.0).astype(np.float32)
    wgr = np.ascontiguousarray(wgr)                     # [h, h']
    bgr = np.asarray(inputs["bg"], np.float32).reshape(H, 1)
    i2e = np.zeros((CH, 2), np.float32)
    i2e[0:HD, 0] = 1.0
    i2e[HD:CH, 1] = 1.0
    i16 = np.zeros((H, D), np.float32)
    for h in range(H):
        i16[h, h * HD:(h + 1) * HD] = 1.0
    i16 = np.ascontiguousarray(
        i16.reshape(H, KT, 128).reshape(H, KT * 128))
    idn = np.eye(128, dtype=BF)
    in_maps = []
    for c in range(NCORES):
        sl = slice(CH * c, CH * c + CH)
        m = {"xT": xT, "wgr": wgr, "bgr": bgr, "i2e": i2e, "i16": i16,
             "idn": idn}
        for nm in ("q", "k", "v", "p"):
            W = np.asarray(inputs[f"W{nm}"], np.float32)
            m["w" + nm] = _tile_rows(W[sl, :].T.astype(BF))
            g = np.asarray(inputs[f"g{nm}"], np.float32)[sl]
            be = np.asarray(inputs[f"beta{nm}"], np.float32)[sl]
            m["t" + nm] = ((2.0 - be) / g).reshape(CH, 1).astype(np.float32)
            m["b" + nm] = np.asarray(
                inputs[f"b{nm}"], np.float32)[sl].reshape(CH, 1)
        in_maps.append(m)
    return in_maps


def _run(inputs, trace=False):
    if "nc" not in _CACHE:
        _CACHE["nc"] = _build()
    nc = _CACHE["nc"]
    in_maps = _prep_inputs(inputs)
    res = run_bass_kernel_spmd(nc, in_maps, core_ids=list(range(NCORES)),
                               trace=trace)
    out = np.empty((TOK, D), np.float32)
    for c in range(NCORES):
        out[:, CH * c:CH * c + CH] = res.results[c]["outT"].astype(np.float32).T
    return out.reshape(B, NSEQ, D), res


def kernel(**inputs) -> np.ndarray:
    out, _ = _run(inputs, trace=False)
    return out



# revision 9
# speedup vs baseline: 1.5667x; 1.5667x over previous
"""Trainium2 Bass kernel for nn_LogicGatedSpikingSelfAttention.

Design (v2): the attention has no softmax, so it is linear:
    x_attn = scale * gate * q @ (k^T v)
which is O(N*hd^2) instead of O(N^2*hd) - a 16x FLOP reduction, with all
intermediate values exact small-integer counts (spikes are {0,1}).

Sharding:
  Stage 1 (channel-parallel): core c owns output channels 128c..128c+127
  (= heads 2c, 2c+1) for the q/k/v branches. BN stats are per-channel
  over all tokens -> fully local. Spikes are thresholded directly from
  PSUM (bias cancels against the threshold shift, so Y is never
  materialized). M_b = k^T v ([128,128] per batch, block-diagonal per
  head) is built locally from PE-transposed spikes; the per-(b,h) logic
  gate folds into the attn-LIF threshold (2^20 when gate=0).
  A tiny (256 B) AllGather shares the per-head energies for the gate.

  Resharding: one 1 MB AllToAll converts attention spikes from
  [own 128 ch, 4096 tok] to [all 1024 ch, own 512 tok].

  Stage 2 (token-parallel): core c owns tokens 512c..512c+511 for the
  projection (full Wp^T resident). BN stats are reduced with an 8 KB
  AllReduce of per-channel (sum, sumsq).

Numerics: spikes/M in fp16 (counts <= 1024 are exact), PSUM f32 exact,
thresholds f32 -> spike decisions bit-match the f32 reference modulo
bf16 rounding of x/W in the dense GEMMs (empirically 0 mismatches).
"""
import numpy as np
import ml_dtypes

import concourse.bass as bass
import concourse.bacc as bacc
import concourse.tile as tile
from concourse import mybir
from concourse.bass_utils import run_bass_kernel_spmd

NCORES = 8
B, NSEQ, D, H = 4, 1024, 1024, 16
HD = D // H            # 64 head dim
CH = D // NCORES       # 128 channels per core (2 heads)
TOK = B * NSEQ         # 4096 tokens
KT = D // 128          # 8 contraction tiles
LTOK = TOK // NCORES   # 512 local tokens for stage 2
EPS = 1e-5
# integer threshold for x_attn spikes: S >= 2^0.75 <=> S >= 1.75 (S integer)
S_TH = 1.75
GATE_OFF = float(2.0 ** 20)          # threshold when gate == 0
F32 = mybir.dt.float32
BF16 = mybir.dt.bfloat16
FP16 = mybir.dt.float16
BF = ml_dtypes.bfloat16

_CACHE = {}


def _build(for_sim=False):
    nc = bacc.Bacc("TRN2", target_bir_lowering=False, debug=False,
                   num_devices=NCORES)
    inp = {}
    def din(name, shape, dt=BF16):
        inp[name] = nc.dram_tensor(name, shape, dt, kind="ExternalInput")
        return inp[name]

    for kt in range(KT):                  # x pre-tiled per kt chunk
        din(f"xt{kt}", [128, TOK])
    din("wq", [128, KT * CH]); din("wk", [128, KT * CH])
    din("wv", [128, KT * CH])
    din("wpT", [128, KT * 8 * 128])       # [p, kt, mt, m] = Wp[mt*128+m, kt*128+p]
    for nm in ("tq", "tk", "tv"):
        din(nm, [CH, 1], F32)
    din("tp", [128, 8], F32)              # (2-beta_p)/gamma_p per (p, mt)
    din("wgr", [H, H], F32)               # lhsT: [h, h'] = sum_r Wg[h', h+16r]/1024
    din("bgr", [H, 1], F32)
    din("i2e", [CH, 2], F32)              # [p, j] = (p//64==j)
    din("sel2", [H, 2], F32)              # per-core: [h, j] = (h == 2c+j)
    din("sel128", [2, 128], F32)          # [j, p] = (p//64 == j)
    din("idn", [128, 128], FP16)          # identity for PE transpose
    outT = nc.dram_tensor("outT", [8, 128, LTOK], BF16, kind="ExternalOutput")

    with tile.TileContext(nc) as tc:
        with tc.tile_pool(name="consts", bufs=1) as consts, \
             tc.tile_pool(name="spikes", bufs=1) as spk_pool, \
             tc.tile_pool(name="dram", bufs=1, space="DRAM") as dram:
            _body(tc, inp, outT, consts, spk_pool, dram)
    if for_sim:
        nc.insert_bir_kernel_barrier_sem_inc()
    else:
        nc.compile()
    return nc


def _body(tc, inp, outT, consts, spk_pool, dram):
    nc = tc.nc
    V, SC, GP, TE = nc.vector, nc.scalar, nc.gpsimd, nc.tensor
    AF = mybir.ActivationFunctionType
    OP = mybir.AluOpType
    DENG = [nc.sync, nc.scalar, nc.gpsimd]

    # ---- DRAM scratch for the collectives ----
    e_pay = dram.tile([2 * B], F32)
    e_gath = dram.tile([NCORES * 2 * B], F32, addr_space="Shared")
    a2a_pay = dram.tile([NCORES * 128 * LTOK], BF16)
    a2a_out = dram.tile([NCORES * 128 * LTOK], BF16)
    st_pay = dram.tile([128 * 8 * 2], F32)
    st_out = dram.tile([128 * 8 * 2], F32, addr_space="Shared")

    # ---- constants / weights to SBUF ----
    small = {}
    for nm in ("tq", "tk", "tv", "bgr"):
        t = consts.tile([inp[nm].shape[0], 1], F32, name=f"{nm}_sb")
        nc.sync.dma_start(t[:], inp[nm].ap())
        small[nm] = t
    tp_sb = consts.tile([128, 8], F32)
    nc.sync.dma_start(tp_sb[:], inp["tp"].ap())
    wgr_sb = consts.tile([H, H], F32)
    nc.sync.dma_start(wgr_sb[:], inp["wgr"].ap())
    i2e_sb = consts.tile([CH, 2], F32)
    nc.sync.dma_start(i2e_sb[:], inp["i2e"].ap())
    sel2_sb = consts.tile([H, 2], F32)
    nc.sync.dma_start(sel2_sb[:], inp["sel2"].ap())
    sel128_sb = consts.tile([2, 128], F32)
    nc.sync.dma_start(sel128_sb[:], inp["sel128"].ap())
    idn_sb = consts.tile([128, 128], FP16)
    nc.scalar.dma_start(idn_sb[:], inp["idn"].ap())
    eps_sb = consts.tile([128, 1], F32)
    V.memset(eps_sb[:], EPS)
    w_sb = {}
    for i, nm in enumerate(("wq", "wk", "wv")):
        t = consts.tile([128, KT, CH], BF16, name=f"{nm}_sb")
        DENG[i % 3].dma_start(
            t[:], inp[nm].ap().rearrange("p (t m) -> p t m", t=KT))
        w_sb[nm] = t
    # x chunks: 8 separate tiles so branch matmuls start after chunk 0
    xts = []
    for kt in range(KT):
        t = consts.tile([128, TOK], BF16, name=f"xt{kt}_sb")
        DENG[kt % 3].dma_start(t[:], inp[f"xt{kt}"].ap())
        xts.append(t)
    # Wp^T is only needed by the projection - load last (same queues)
    wpT_sb = consts.tile([128, KT, 8, 128], BF16)
    for kt in range(KT):
        DENG[kt % 3].dma_start(
            wpT_sb[:, kt, :, :],
            inp["wpT"].ap().rearrange("p (t m n) -> p t m n",
                                      t=KT, m=8)[:, kt, :, :])

    # ---- persistent spike tensors (fp16: {0,1} exact) ----
    sp = {nm: spk_pool.tile([128, TOK], FP16, name=f"sp{nm}")
          for nm in ("q", "k", "v")}
    knat = spk_pool.tile([128, 32, 128], FP16)   # [tok, b*8+t, ch]
    vnat = spk_pool.tile([128, 32, 128], FP16)
    m_sb = spk_pool.tile([128, B, 128], FP16)    # block-diag k^T v per batch
    V.memset(m_sb[:], 0.0)
    xsp_cm = spk_pool.tile([128, TOK], BF16)     # gated attn spikes (ch-major)
    xsp_tok = spk_pool.tile([128, KT, LTOK], BF16)  # after AllToAll (tok-shard)

    # ================= branches (q, k, v) =================
    with tc.tile_pool(name="stps", bufs=2) as stp:
        for nm in ("q", "k", "v"):
            with tc.tile_pool(name=f"brps_{nm}", bufs=1, space="PSUM") as brps:
                ps = [brps.tile([128, 512], F32, name=f"ps{nm}{i}")
                      for i in range(8)]
                for kt in range(KT):
                    for nck in range(8):
                        TE.matmul(ps[nck][:], w_sb["w" + nm][:, kt, :],
                                  xts[kt][:, nck * 512:(nck + 1) * 512],
                                  start=(kt == 0), stop=(kt == KT - 1))
                # stats + threshold directly on PSUM (bias cancels)
                stats = stp.tile([128, 8, 6], F32, tag="stats")
                for i in range(8):
                    V.bn_stats(stats[:, i, :], ps[i][:])
                mv = stp.tile([128, 2], F32, tag="mv")
                V.bn_aggr(mv[:], stats[:])
                std = stp.tile([128, 1], F32, tag="std")
                SC.activation(std[:], mv[:, 1:2], AF.Sqrt, bias=eps_sb[:])
                thr = stp.tile([128, 1], F32, tag="thr")
                V.tensor_tensor(thr[:], std[:], small["t" + nm][:], OP.mult)
                V.tensor_tensor(thr[:], thr[:], mv[:, 0:1], OP.add)
                for i in range(8):
                    V.tensor_scalar(sp[nm][:, i * 512:(i + 1) * 512],
                                    ps[i][:], thr[:], None, OP.is_ge)

            if nm == "k":
                # energies for the logic gate: counts of q&k per (head, b)
                with tc.tile_pool(name="enps", bufs=1, space="PSUM") as enps, \
                     tc.tile_pool(name="entmp", bufs=1) as entmp:
                    prod = entmp.tile([128, TOK], FP16)
                    GP.tensor_tensor(prod[:], sp["q"][:], sp["k"][:], OP.mult)
                    ech = entmp.tile([128, B], F32)
                    V.reduce_sum(ech[:],
                                 prod[:].rearrange("p (b n) -> p b n", b=B),
                                 axis=mybir.AxisListType.X)
                    e_ps = enps.tile([2, B], F32)
                    TE.matmul(e_ps[:], i2e_sb[:], ech[:], start=True, stop=True)
                    e_sb = entmp.tile([2, B], F32)
                    SC.activation(e_sb[:], e_ps[:], AF.Copy)
                    nc.sync.dma_start(
                        e_pay[:].rearrange("(p w) -> p w", p=2), e_sb[:])
                GP.collective_compute(
                    "AllGather", OP.bypass,
                    ins=[e_pay.opt()], outs=[e_gath.opt()],
                    replica_groups=[list(range(NCORES))])

    # ================= gate -> per-batch attn-LIF thresholds =================
    gtmp = spk_pool  # small persistent tiles
    thrv = gtmp.tile([128, B], F32)
    with tc.tile_pool(name="gtps", bufs=1, space="PSUM") as gtps, \
         tc.tile_pool(name="gsb", bufs=1) as gsb:
        eg = gsb.tile([H, B], F32)
        nc.sync.dma_start(eg[:], e_gath[:].rearrange("(h b) -> h b", h=H))
        z_ps = gtps.tile([H, B], F32)
        TE.matmul(z_ps[:], wgr_sb[:], eg[:], start=True, stop=True)
        gate = gsb.tile([H, B], F32)
        V.tensor_scalar(gate[:], z_ps[:], small["bgr"][:], 0.5,
                        OP.add, OP.is_ge)
        # {0,1} -> {GATE_OFF, S_TH} (exact in f32)
        gthr = gsb.tile([H, B], F32)
        V.tensor_scalar(gthr[:], gate[:], S_TH - GATE_OFF, GATE_OFF,
                        OP.mult, OP.add)
        g2_ps = gtps.tile([2, B], F32)
        TE.matmul(g2_ps[:], sel2_sb[:], gthr[:], start=True, stop=True)
        g2 = gsb.tile([2, B], F32)
        V.tensor_copy(g2[:], g2_ps[:])
        tv_ps = gtps.tile([128, B], F32)
        TE.matmul(tv_ps[:], sel128_sb[:], g2[:], start=True, stop=True)
        V.tensor_copy(thrv[:], tv_ps[:])

    # ========== attention: transpose k,v -> M = k^T v -> apply ==========
    with tc.tile_pool(name="tps", bufs=4, space="PSUM") as tps, \
         tc.tile_pool(name="mps", bufs=1, space="PSUM") as mps:
        for b in range(B):
            for i in range(8):
                for j, (src, dst) in enumerate(((sp["k"], knat),
                                               (sp["v"], vnat))):
                    t = tps.tile([128, 128], FP16, tag="t")
                    TE.transpose(t[:], src[:, b * NSEQ + i * 128:
                                            b * NSEQ + (i + 1) * 128], idn_sb[:])
                    if (i * 2 + j) % 2 == 0:
                        V.tensor_copy(dst[:, b * 8 + i, :], t[:])
                    else:
                        SC.activation(dst[:, b * 8 + i, :], t[:], AF.Copy)
            m_ps = mps.tile([128, 128], F32, name=f"mps{b}")
            for i in range(8):
                TE.matmul(m_ps[:], knat[:, b * 8 + i, :], vnat[:, b * 8 + i, :],
                          start=(i == 0), stop=(i == 7))
            # keep only the per-head diagonal 64-blocks (fp16 exact: <=1024)
            V.tensor_copy(m_sb[0:64, b, 0:64], m_ps[0:64, 0:64])
            SC.activation(m_sb[64:128, b, 64:128], m_ps[64:128, 64:128], AF.Copy)

    # apply: S = M_b^T q  (exact integer counts), gate-folded threshold
    with tc.tile_pool(name="aps", bufs=1, space="PSUM") as aps:
        for b in range(B):
            for nh in range(2):
                n0 = b * NSEQ + nh * 512
                s_ps = aps.tile([128, 512], F32, name=f"s{b}{nh}")
                TE.matmul(s_ps[:], m_sb[:, b, :], sp["q"][:, n0:n0 + 512],
                          start=True, stop=True)
                V.tensor_scalar(xsp_cm[:, n0:n0 + 512], s_ps[:],
                                thrv[:, b:b + 1], None, OP.is_ge)

    # ================= AllToAll: reshard to token-parallel =================
    for j in range(NCORES):
        DENG[j % 3].dma_start(
            a2a_pay[j * 128 * LTOK:(j + 1) * 128 * LTOK].rearrange(
                "(p n) -> p n", p=128),
            xsp_cm[:, j * LTOK:(j + 1) * LTOK])
    GP.collective_compute("AllToAll", OP.bypass,
                          ins=[a2a_pay.opt()], outs=[a2a_out.opt()],
                          replica_groups=[list(range(NCORES))])
    for j in range(NCORES):
        DENG[j % 3].dma_start(
            xsp_tok[:, j, :],
            a2a_out[j * 128 * LTOK:(j + 1) * 128 * LTOK].rearrange(
                "(p n) -> p n", p=128))

    # ================= projection (token-parallel) =================
    with tc.tile_pool(name="pstat", bufs=1) as pstat:
        with tc.tile_pool(name="ppps", bufs=1, space="PSUM") as ppps:
            pp = [ppps.tile([128, LTOK], F32, name=f"pp{i}") for i in range(8)]
            for kt in range(KT):
                for mt in range(8):
                    TE.matmul(pp[mt][:], wpT_sb[:, kt, mt, :], xsp_tok[:, kt, :],
                              start=(kt == 0), stop=(kt == KT - 1))
            # per-channel partial stats over the 512 local tokens
            stats = pstat.tile([128, 8, 6], F32)
            mv8 = pstat.tile([128, 8, 2], F32)
            for mt in range(8):
                V.bn_stats(stats[:, mt, :], pp[mt][:])
                V.bn_aggr(mv8[:, mt, :], stats[:, mt, :])
            s12 = pstat.tile([128, 8, 2], F32)
            # s1 = 512*mean ; s2 = 512*(var + mean^2)
            V.tensor_scalar(s12[:, :, 0], mv8[:, :, 0], float(LTOK), None,
                            OP.mult)
            msq = pstat.tile([128, 8], F32)
            V.tensor_tensor(msq[:], mv8[:, :, 0], mv8[:, :, 0], OP.mult)
            V.tensor_tensor(msq[:], msq[:], mv8[:, :, 1], OP.add)
            V.tensor_scalar(s12[:, :, 1], msq[:], float(LTOK), None, OP.mult)
            nc.sync.dma_start(
                st_pay[:].rearrange("(p w) -> p w", p=128), s12[:])
            GP.collective_compute("AllReduce", OP.add,
                                  ins=[st_pay.opt()], outs=[st_out.opt()],
                                  replica_groups=[list(range(NCORES))])
            g12 = pstat.tile([128, 8, 2], F32)
            nc.sync.dma_start(
                g12[:], st_out[:].rearrange("(p m w) -> p m w", p=128, m=8))
            meang = pstat.tile([128, 8], F32)
            V.tensor_scalar(meang[:], g12[:, :, 0], 1.0 / TOK, None, OP.mult)
            varg = pstat.tile([128, 8], F32)
            V.tensor_scalar(varg[:], g12[:, :, 1], 1.0 / TOK, None, OP.mult)
            msq2 = pstat.tile([128, 8], F32)
            V.tensor_tensor(msq2[:], meang[:], meang[:], OP.mult)
            V.tensor_tensor(varg[:], varg[:], msq2[:], OP.subtract)
            stdp = pstat.tile([128, 8], F32)
            SC.activation(stdp[:], varg[:], AF.Sqrt, bias=eps_sb[:])
            thrp = pstat.tile([128, 8], F32)
            V.tensor_tensor(thrp[:], stdp[:], tp_sb[:], OP.mult)
            V.tensor_tensor(thrp[:], thrp[:], meang[:], OP.add)
            osp = pstat.tile([128, 8, LTOK], BF16)
            for mt in range(8):
                V.tensor_scalar(osp[:, mt, :], pp[mt][:],
                                thrp[:, mt:mt + 1], None, OP.is_ge)
                DENG[mt % 3].dma_start(outT.ap()[mt, :, :], osp[:, mt, :])


def _tile_rows(a):
    # (8*128, N) -> (128, 8*N) so the SBUF [p, (t n)] load is contiguous
    n = a.shape[1]
    return np.ascontiguousarray(
        a.reshape(KT, 128, n).transpose(1, 0, 2).reshape(128, KT * n))


def _prep_inputs(inputs):
    x = np.asarray(inputs["x"], np.float32)
    xT = _tile_rows(x.reshape(TOK, D).T.astype(BF))
    Wg = np.asarray(inputs["Wg"], np.float64)
    wgr = (Wg.reshape(H, HD, H).sum(axis=1).T / 1024.0).astype(np.float32)
    wgr = np.ascontiguousarray(wgr)                     # [h, h']
    bgr = np.asarray(inputs["bg"], np.float32).reshape(H, 1)
    i2e = np.zeros((CH, 2), np.float32)
    i2e[0:HD, 0] = 1.0
    i2e[HD:CH, 1] = 1.0
    sel128 = np.zeros((2, 128), np.float32)
    sel128[0, 0:64] = 1.0
    sel128[1, 64:128] = 1.0
    idn = np.eye(128, dtype=np.float16)
    # Wp^T tiled: wpT[p, kt, mt, m] = Wp[mt*128+m, kt*128+p]
    Wp = np.asarray(inputs["Wp"], np.float32).astype(BF)
    wpT = np.ascontiguousarray(
        Wp.reshape(8, 128, 8, 128).transpose(3, 2, 0, 1)).reshape(128, -1)
    gpf = np.asarray(inputs["gp"], np.float32)
    bepf = np.asarray(inputs["betap"], np.float32)
    tpv = (2.0 - bepf) / gpf                            # [1024] per c_out
    tp = np.ascontiguousarray(tpv.reshape(8, 128).T).astype(np.float32)
    in_maps = []
    for c in range(NCORES):
        sl = slice(CH * c, CH * c + CH)
        sel2 = np.zeros((H, 2), np.float32)
        sel2[2 * c, 0] = 1.0
        sel2[2 * c + 1, 1] = 1.0
        m = {"wgr": wgr, "bgr": bgr, "i2e": i2e, "sel2": sel2,
             "sel128": sel128, "idn": idn, "wpT": wpT, "tp": tp}
        for kt in range(KT):
            m[f"xt{kt}"] = np.ascontiguousarray(xT[:, kt * TOK:(kt + 1) * TOK])
        for nm in ("q", "k", "v"):
            W = np.asarray(inputs[f"W{nm}"], np.float32)
            m["w" + nm] = _tile_rows(W[sl, :].T.astype(BF))
            g = np.asarray(inputs[f"g{nm}"], np.float32)[sl]
            be = np.asarray(inputs[f"beta{nm}"], np.float32)[sl]
            m["t" + nm] = ((2.0 - be) / g).reshape(CH, 1).astype(np.float32)
        in_maps.append(m)
    return in_maps


def _assemble(results):
    out = np.empty((TOK, D), np.float32)
    for c in range(NCORES):
        o = np.asarray(results[c]["outT"], dtype=np.float32)  # [8, 128, 512]
        out[LTOK * c:LTOK * (c + 1), :] = \
            o.transpose(2, 0, 1).reshape(LTOK, D)
    return out.reshape(B, NSEQ, D)


def _run(inputs, trace=False):
    if "nc" not in _CACHE:
        _CACHE["nc"] = _build()
    nc = _CACHE["nc"]
    in_maps = _prep_inputs(inputs)
    res = run_bass_kernel_spmd(nc, in_maps, core_ids=list(range(NCORES)),
                               trace=trace)
    return _assemble(res.results), res


def kernel(**inputs) -> np.ndarray:
    out, _ = _run(inputs, trace=False)
    return out
